# revision 30
# baseline (speedup 1.0000x reference)
"""AppendVarGLCM Trainium2 kernel (8 NeuronCores, SPMD).

out = concat([image, var[None]], axis=0), var = variance over the 4
skimage-style d=1 GLCM angle histograms of the u8-quantized band image[index].

Per-core work:
  - full band (256x256) -> u8 quantization (redundant on every core):
    DVE min/max reduce, then a PE-transpose / reduce / broadcast-matmul
    chain for the cross-partition min/max (cheaper than gpsimd
    partition_all_reduce), exact rescale, round-half-even via the fp32
    magic-constant trick (matches jnp.round).
  - the u8 band is written into a 258-wide sentinel-PADDED layout already
    in SBUF ([128, 2, 258], sentinel=300 in the two pad columns), so the
    DRAM staging round-trip is two fully contiguous DMAs (write at a
    core-dependent shift, fixed-window read back) instead of hundreds of
    row descriptors.  Small head/tail sentinel prefills cover the halo
    ends.  After the shift, the 4 GLCM neighbor offsets are uniform
    linear shifts (+1, +259, +258, +257) and sentinel positions one-hot
    to all-zero rows.
  - GLCM counts as one-hot outer-product matmuls on the TensorEngine in
    fp8e4 DoubleRow mode.  One-hots are built as INT16 (is_equal * 56 =
    0x0038, whose low byte is fp8e4 1.0), which keeps every DVE operand
    2-byte so the builds run in 4x DVE perf mode; the matmuls read the
    same bytes through a stride-2 float8e4 bitcast view.  A 135-slot
    one-hot stream buffer builds each distinct halo column once.
  - fp16 partial histograms ([128, 2048] = 512KB) ReduceScattered across
    the 8 cores (counts stay far below 2048, so fp16 sums are exact);
    per-core variance over angles for its 1/8 of bins -> [16, 512].
    Squares on the Act engine overlap the sum chain on DVE.
  - The 5.9MB image-plane copy (DRAM->DRAM) starts right after the halo
    read and fully overlaps the GLCM loop, finishing before the
    collective needs the DMA rings.
"""
import sys

for _p in ("/opt/trn_rl_repo",):
    if _p not in sys.path:
        sys.path.insert(0, _p)

import numpy as np

import concourse.bass as bass
import concourse.mybir as mybir
from concourse import bacc, tile
from concourse.bass_utils import run_bass_kernel_spmd
from concourse.tile_rust import add_dep_helper

F32 = mybir.dt.float32
F16 = mybir.dt.float16
FP8 = mybir.dt.float8e4
I16 = mybir.dt.int16

N_CORES = 8
NPLANES = 180
H = W = 256
ROWS_PER_CORE = NPLANES * H // N_CORES  # 5760

PW = 258                  # padded row width
TCOLS = 528               # pair columns: 128 * 528 = 67584 >= 258*258
TPC = TCOLS // N_CORES    # 66 pair columns per core
RD_BASE = 462             # fixed halo read base; write base = 721 - 66*m
STG = 69120               # staging elements (= RD_BASE + 128*TCOLS + pad)
SENT = 300.0

_CACHED = {}


def build_nc():
    nc = bacc.Bacc("TRN2", target_bir_lowering=False, debug=False,
                   enable_asserts=False, num_devices=N_CORES)

    img = nc.declare_dram_parameter("img", [ROWS_PER_CORE, 256], F32,
                                    isOutput=False)
    band = nc.declare_dram_parameter("band", [128, 512], F32, isOutput=False)
    img_out = nc.declare_dram_parameter("img_out", [ROWS_PER_CORE, 256], F32,
                                        isOutput=True)
    var_out = nc.declare_dram_parameter("var_out", [128, 512], F32,
                                        isOutput=True)

    staging = nc.dram_tensor("staging", [STG], F32)
    cc_in = nc.dram_tensor("cc_in", [128 * 2048], F16)
    # AllReduce (not ReduceScatter) because only AllReduce supports a
    # "Shared" output scratchpad, which is the collective fast path; every
    # core then computes the full variance plane (same per-op DVE cost --
    # partitions are parallel) and the host uses core 0's copy.
    cc_out = nc.dram_tensor("cc_out", [128 * 2048], F16, addr_space="Shared")


    with tile.TileContext(nc) as tc:
        with (
            tc.tile_pool(name="const", bufs=1) as cpool,
            tc.tile_pool(name="prep", bufs=1) as prep,
            tc.tile_pool(name="psum", bufs=1, space="PSUM") as psp,
            tc.tile_pool(name="post", bufs=1) as post,
        ):
            stg_flat = staging.ap()

            # ---- dependency-free constants / prefills ----
            band_t = prep.tile([128, 512], F32)
            nc.sync.dma_start(out=band_t[0:64, :], in_=band[0:64, :])
            nc.gpsimd.dma_start(out=band_t[64:128, :], in_=band[64:128, :])

            sent4 = prep.tile([4, 736], F32)
            nc.vector.memset(sent4[:], SENT)
            # head [0, 736) and tail [66176, 69120) sentinel prefill; the
            # pixel write (base = 721-66m, 66048 long) lands inside and
            # overwrites the middle.
            nc.scalar.dma_start(out=stg_flat[0:736].rearrange(
                "(p f) -> p f", p=1), in_=sent4[0:1, :])
            nc.scalar.dma_start(out=stg_flat[66176:69120].rearrange(
                "(p f) -> p f", p=4), in_=sent4[:, :])

            iota16 = cpool.tile([128, 256], I16)
            nc.gpsimd.iota(iota16[:], pattern=[[1, 256]], base=0,
                           channel_multiplier=0)
            pid16 = cpool.tile([128, 1], I16)
            nc.gpsimd.iota(pid16[:], pattern=[[1, 1]], base=0,
                           channel_multiplier=1)
            pidf = cpool.tile([128, 1], F32)
            nc.vector.tensor_copy(pidf[:], pid16[:])
            ident = cpool.tile([128, 128], F32)
            nc.vector.tensor_scalar(ident[:], iota16[:, 0:128], pidf[:],
                                    None, mybir.AluOpType.is_equal)
            ones1 = cpool.tile([1, 128], F32)
            nc.vector.memset(ones1[:], 1.0)

            # ---- quantize band to u8 (identical on every core) ----
            mn = prep.tile([128, 1], F32)
            mx2 = prep.tile([128, 2], F32)  # [:,0] = -lo_p, [:,1] = hi_p
            nc.vector.tensor_reduce(mn[:], band_t[:], mybir.AxisListType.X,
                                    mybir.AluOpType.min)
            nc.vector.tensor_reduce(mx2[:, 1:2], band_t[:],
                                    mybir.AxisListType.X, mybir.AluOpType.max)
            nc.vector.tensor_scalar(mx2[:, 0:1], mn[:], -1.0, None,
                                    mybir.AluOpType.mult)
            # cross-partition max via PE transpose -> reduce -> transpose
            # -> ones-matmul broadcast
            psQ = psp.tile([128, 512], F32, name="psQ", tag="psQ")
            nc.tensor.transpose(psQ[0:2, 0:128], mx2[:], ident[:])
            sT = prep.tile([2, 128], F32)
            nc.scalar.copy(sT[:], psQ[0:2, 0:128])
            t2 = prep.tile([2, 1], F32)
            nc.vector.tensor_reduce(t2[:], sT[:], mybir.AxisListType.X,
                                    mybir.AluOpType.max)
            nc.tensor.transpose(psQ[0:1, 128:130], t2[:], ident[0:2, 0:2])
            sT2 = prep.tile([1, 2], F32)
            nc.vector.tensor_copy(sT2[:], psQ[0:1, 128:130])
            nc.tensor.matmul(psQ[:, 130:132], ones1[:], sT2[:], start=True,
                             stop=True)
            pmax = prep.tile([128, 2], F32)  # [:,0] = -lo, [:,1] = hi
            nc.vector.tensor_copy(pmax[:], psQ[:, 130:132])

            den = prep.tile([128, 1], F32)
            nc.vector.tensor_tensor(den[:], pmax[:, 1:2], pmax[:, 0:1],
                                    mybir.AluOpType.add)  # hi - lo
            nc.vector.tensor_scalar(den[:], den[:], 1e-12, None,
                                    mybir.AluOpType.max)
            rcp = prep.tile([128, 1], F32)
            nc.vector.reciprocal(rcp[:], den[:])
            nc.vector.tensor_scalar(rcp[:], rcp[:], 255.0, None,
                                    mybir.AluOpType.mult)
            scaled = prep.tile([128, 512], F32)
            nc.vector.tensor_scalar(scaled[:], band_t[:], pmax[:, 0:1], None,
                                    mybir.AluOpType.add)      # band - lo
            nc.vector.tensor_scalar(scaled[:], scaled[:], rcp[:], None,
                                    mybir.AluOpType.mult)     # * 255/(hi-lo)

            # ---- u8 quantize straight into the SBUF-padded layout ----
            u8p = prep.tile([128, 516], F32)  # [128, 2, 258] padded rows
            u8v = u8p[:].rearrange("p (k c) -> p k c", c=PW)
            nc.vector.memset(u8v[:, :, 256:258], SENT)
            # round-to-nearest-even via the fp32 magic constant: for
            # 0 <= x < 2^22, (x + 1.5*2^23) - 1.5*2^23 == round(x)
            MAGIC = 12582912.0
            nc.vector.tensor_scalar(u8v[:, :, 0:256], scaled[:], MAGIC,
                                    -MAGIC, mybir.AluOpType.add,
                                    mybir.AluOpType.add)

            # ---- staging: static contiguous write, static overlapped-row
            # read, per-core column shift as a register-offset DVE copy ----
            # write pixels at stg[259 + 258r + c]; core m's halo column j of
            # partition p is stg[528p + 66m + j], so a static [128, 990]
            # read (partition stride 528, overlapping rows) covers all
            # cores and the shift is a 66m column offset in SBUF.
            wr2d = stg_flat[259:259 + 128 * 516].rearrange(
                "(p f) -> p f", f=516)
            for eng, lo, hi in ((nc.sync, 0, 48), (nc.scalar, 48, 88),
                                (nc.gpsimd, 88, 128)):
                eng.dma_start(out=wr2d[lo:hi, :], in_=u8p[lo:hi, :])
            # two 2D reads with width <= pitch (the 990-wide window's rows
            # overlap by 462, which would break 2D-descriptor mode), each
            # split across two queues
            wide = prep.tile([128, 990], F32)
            rd_dmas = [
                nc.sync.dma_start(
                    out=wide[0:64, 0:528],
                    in_=bass.AP(stg_flat.tensor, 0, [[528, 64], [1, 528]])),
                nc.scalar.dma_start(
                    out=wide[64:128, 0:528],
                    in_=bass.AP(stg_flat.tensor, 528 * 64,
                                [[528, 64], [1, 528]])),
                nc.gpsimd.dma_start(
                    out=wide[:, 528:990],
                    in_=bass.AP(stg_flat.tensor, 528,
                                [[528, 128], [1, 462]])),
            ]
            pidv = nc.vector.partition_id()
            halo = prep.tile([128, 528], F32)
            nc.vector.tensor_copy(halo[:], wide[:, bass.ds(TPC * pidv, 528)])

            # ---- big image copy (DRAM -> DRAM) ----
            # Held back until the halo read completes so the staging chain
            # never queues behind it; it then overlaps the GLCM loop and
            # drains before the collective needs the DMA rings.
            chunk = ROWS_PER_CORE // 4
            for c in range(4):
                eng = nc.scalar if c < 2 else nc.gpsimd
                cp = eng.dma_start(
                    out=img_out[c * chunk:(c + 1) * chunk, :],
                    in_=img[c * chunk:(c + 1) * chunk, :],
                )
                for rd in rd_dmas:
                    add_dep_helper(cp.ins, rd.ins, sync=True,
                                   reason="image copy after GLCM prep DMAs")

            # ---- GLCM one-hot matmuls (fp8 DoubleRow; fp16 build trick) ----
            # stream[:, s, :] (fp16) = (iota == halo col(s)): col(s) = s for
            # s <= 66 (A/B0 roles), col(s) = s+190 for s >= 67 (cols
            # 257..324, B1/B2/B3 roles).  fp16 1.0 = 0x3C00, so through the
            # fp8 bitcast view the ODD bytes are a one-hot with value 1.5
            # (e4m3 0x3C); products come out scaled by 2.25, undone at the
            # PSUM drain.  Keeping every DVE operand 2-byte and the op
            # single-ALU gives the builds a shot at 4x DVE perf mode.
            # Roles per column t: A=slot t, B0=t+1, B3=t+67, B2=t+68,
            # B1=t+69.
            DR = mybir.MatmulPerfMode.DoubleRow
            ps0 = psp.tile([128, 512], F32, name="ps0", tag="ps0")
            ps1a = psp.tile([128, 256], F32, name="ps1a", tag="ps1a")
            ps1b = psp.tile([128, 256], F32, name="ps1b", tag="ps1b")
            ps2 = psp.tile([128, 512], F32, name="ps2", tag="ps2")
            ps3a = psp.tile([128, 256], F32, name="ps3a", tag="ps3a")
            ps3b = psp.tile([128, 256], F32, name="ps3b", tag="ps3b")
            stream = cpool.tile([128, 135, 256], F16)

            def build(slot, col):
                nc.vector.tensor_scalar(
                    stream[:, slot, :], iota16[:], halo[:, col:col + 1],
                    None, mybir.AluOpType.is_equal)

            # Act-engine build (offloads 1 of 4 per-iteration builds from
            # the DVE, which is the loop bottleneck): relu(1 - |halo - j|)
            # is 1.0 exactly at the matching level, 0 elsewhere.
            AF = mybir.ActivationFunctionType
            tmp_b = cpool.tile([128, 256], F16)

            def build_act(slot, col):
                nc.scalar.activation(tmp_b[:], iota16[:], AF.Abs,
                                     bias=halo[:, col:col + 1], scale=-1.0)
                nc.scalar.activation(stream[:, slot, :], tmp_b[:], AF.Relu,
                                     bias=1.0, scale=-1.0)

            build(0, 0)
            build(67, 257)
            build(68, 258)
            st8 = stream[:].bitcast(FP8)  # [128, 135, 512] fp8 view
            pd8 = list(st8.ap[0])
            t8 = st8.tensor

            def mk(slot0, off, n):
                # [K = 128p x 2 ktiles, N = n] stride-2 odd-byte view;
                # ktile k is slot slot0+k.
                return bass.AP(t8, st8.offset + slot0 * 512 + off + 1,
                               [pd8, [512, 2], [2, n]])

            for tt in range(0, TPC, 2):
                build(tt + 1, tt + 1)
                build(tt + 2, tt + 2)
                build(tt + 69, tt + 259)
                build_act(tt + 70, tt + 260)
                st, sp = (tt == 0), (tt == TPC - 2)
                a_lo = mk(tt, 0, 128)
                a_hi = mk(tt, 256, 128)
                r32 = mk(tt + 67, 0, 512)   # [B3 | B2] for pairs (t, t+1)
                rb1 = mk(tt + 69, 0, 256)
                rb0 = mk(tt + 1, 0, 256)
                nc.tensor.matmul(ps0[:], a_lo, r32, start=st, stop=sp,
                                 perf_mode=DR)
                nc.tensor.matmul(ps1a[:], a_lo, rb1, start=st, stop=sp,
                                 perf_mode=DR)
                nc.tensor.matmul(ps1b[:], a_lo, rb0, start=st, stop=sp,
                                 perf_mode=DR)
                nc.tensor.matmul(ps2[:], a_hi, r32, start=st, stop=sp,
                                 perf_mode=DR)
                nc.tensor.matmul(ps3a[:], a_hi, rb1, start=st, stop=sp,
                                 perf_mode=DR)
                nc.tensor.matmul(ps3b[:], a_hi, rb0, start=st, stop=sp,
                                 perf_mode=DR)

            # ---- counts -> DRAM -> ReduceScatter ----
            # counts_sb[l, 1024*h + 256*k + j] = counts[level 128*h+l, j, k];
            # the 1/2.25 undoes the 1.5*1.5 one-hot product scale.
            DS = 1.0 / 2.25
            counts_sb = post.tile([128, 2048], F16)
            nc.scalar.mul(counts_sb[:, 0:512], ps0[:], DS)
            nc.vector.tensor_scalar(counts_sb[:, 512:768], ps1a[:], DS, None,
                                    mybir.AluOpType.mult)
            nc.vector.tensor_scalar(counts_sb[:, 768:1024], ps1b[:], DS, None,
                                    mybir.AluOpType.mult)
            nc.scalar.mul(counts_sb[:, 1024:1536], ps2[:], DS)
            nc.vector.tensor_scalar(counts_sb[:, 1536:1792], ps3a[:], DS,
                                    None, mybir.AluOpType.mult)
            nc.vector.tensor_scalar(counts_sb[:, 1792:2048], ps3b[:], DS,
                                    None, mybir.AluOpType.mult)
            cc2d = cc_in.ap().rearrange("(p f) -> p f", p=128)
            nc.sync.dma_start(out=cc2d[0:64, :], in_=counts_sb[0:64, :])
            nc.scalar.dma_start(out=cc2d[64:128, :], in_=counts_sb[64:128, :])
            nc.gpsimd.collective_compute(
                "AllReduce",
                mybir.AluOpType.add,
                replica_groups=[list(range(N_CORES))],
                ins=[cc_in.ap().opt()],
                outs=[cc_out.ap().opt()],
            )
            co2d = cc_out.ap().rearrange("(p f) -> p f", p=128)
            c16 = post.tile([128, 2048], F16)
            nc.sync.dma_start(out=c16[0:64, :], in_=co2d[0:64, :])
            nc.scalar.dma_start(out=c16[64:128, :], in_=co2d[64:128, :])

            # ---- variance over the 4 angles (full plane on every core) ----
            # squares on the Act engine overlap the sum chain on DVE
            c3 = c16[:].rearrange("p (h k j) -> p h k j", h=2, k=4)
            csq = post.tile([128, 2048], F32)
            q3 = csq[:].rearrange("p (h k j) -> p h k j", h=2, k=4)
            nc.scalar.square(csq[:], c16[:])
            s = post.tile([128, 512], F32)
            q = post.tile([128, 512], F32)
            s2 = s[:].rearrange("p (h j) -> p h j", h=2)
            q2 = q[:].rearrange("p (h j) -> p h j", h=2)
            nc.vector.tensor_tensor(s2[:, :, :], c3[:, :, 0, :],
                                    c3[:, :, 1, :], mybir.AluOpType.add)
            nc.vector.tensor_tensor(s2[:, :, :], s2[:, :, :], c3[:, :, 2, :],
                                    mybir.AluOpType.add)
            nc.vector.tensor_tensor(s2[:, :, :], s2[:, :, :], c3[:, :, 3, :],
                                    mybir.AluOpType.add)
            nc.vector.tensor_tensor(q2[:, :, :], q3[:, :, 0, :],
                                    q3[:, :, 1, :], mybir.AluOpType.add)
            nc.vector.tensor_tensor(q2[:, :, :], q2[:, :, :], q3[:, :, 2, :],
                                    mybir.AluOpType.add)
            nc.vector.tensor_tensor(q2[:, :, :], q2[:, :, :], q3[:, :, 3, :],
                                    mybir.AluOpType.add)
            # var = q/4 - (s/16)*s
            tmp = post.tile([128, 512], F32)
            nc.vector.scalar_tensor_tensor(tmp[:], s[:], 0.0625, s[:],
                                           mybir.AluOpType.mult,
                                           mybir.AluOpType.mult)
            var_t = post.tile([128, 512], F32)
            nc.vector.scalar_tensor_tensor(var_t[:], q[:], 0.25, tmp[:],
                                           mybir.AluOpType.mult,
                                           mybir.AluOpType.subtract)
            nc.sync.dma_start(out=var_out[0:64, :], in_=var_t[0:64, :])
            nc.scalar.dma_start(out=var_out[64:128, :], in_=var_t[64:128, :])

    nc.compile()
    return nc


def get_nc():
    if "nc" not in _CACHED:
        _CACHED["nc"] = build_nc()
    return _CACHED["nc"]


def make_in_maps(image, band):
    flat = image.reshape(NPLANES * H, W)
    band2 = np.ascontiguousarray(band.reshape(128, 512))
    return [
        {
            "img": np.ascontiguousarray(
                flat[m * ROWS_PER_CORE:(m + 1) * ROWS_PER_CORE]),
            "band": band2,
        }
        for m in range(N_CORES)
    ]


def assemble(image_shards, var_shards):
    """image_shards: 8 x [5760,256]; var_shards: [128,512] (every core
    computes the full variance plane; core 0's copy is used)."""
    out = np.empty((NPLANES + 1, H, W), dtype=np.float32)
    out[:NPLANES] = np.concatenate(image_shards, axis=0).reshape(NPLANES, H, W)
    v = var_shards[0]
    out[NPLANES, 0:128, :] = v[:, 0:256]
    out[NPLANES, 128:256, :] = v[:, 256:512]
    return out


def kernel(image, index):
    image = np.ascontiguousarray(np.asarray(image, dtype=np.float32))
    idx = int(np.asarray(index))
    band = image[idx]

    nc = get_nc()
    in_maps = make_in_maps(image, band)
    last_err = None
    for attempt in range(3):
        try:
            res = run_bass_kernel_spmd(nc, in_maps,
                                       core_ids=list(range(N_CORES)))
            break
        except Exception as e:  # transient NRT device errors
            last_err = e
            import time
            time.sleep(15)
    else:
        raise last_err
    return assemble(
        [res.results[m]["img_out"] for m in range(N_CORES)],
        [res.results[m]["var_out"] for m in range(N_CORES)],
    )


# revision 38
# speedup vs baseline: 1.2329x; 1.2329x over previous
"""AppendVarGLCM Trainium2 kernel (8 NeuronCores, SPMD).

out = concat([image, var[None]], axis=0), var = variance over the 4
skimage-style d=1 GLCM angle histograms of the u8-quantized band image[index].

Per-core work:
  - full band (256x256) -> u8 quantization (redundant on every core):
    DVE min/max reduce, then a PE-transpose / reduce / broadcast-matmul
    chain for the cross-partition min/max (cheaper than gpsimd
    partition_all_reduce), exact rescale, round-half-even via the fp32
    magic-constant trick (matches jnp.round).
  - the u8 band is written into a 258-wide sentinel-PADDED layout already
    in SBUF ([128, 2, 258], sentinel=300 in the two pad columns), so the
    DRAM staging round-trip is two fully contiguous DMAs (write at a
    core-dependent shift, fixed-window read back) instead of hundreds of
    row descriptors.  Small head/tail sentinel prefills cover the halo
    ends.  After the shift, the 4 GLCM neighbor offsets are uniform
    linear shifts (+1, +259, +258, +257) and sentinel positions one-hot
    to all-zero rows.
  - GLCM counts as one-hot outer-product matmuls on the TensorEngine in
    fp8e4 DoubleRow mode.  One-hots are built as INT16 (is_equal * 56 =
    0x0038, whose low byte is fp8e4 1.0), which keeps every DVE operand
    2-byte so the builds run in 4x DVE perf mode; the matmuls read the
    same bytes through a stride-2 float8e4 bitcast view.  A 135-slot
    one-hot stream buffer builds each distinct halo column once.
  - fp16 partial histograms ([128, 2048] = 512KB) ReduceScattered across
    the 8 cores (counts stay far below 2048, so fp16 sums are exact);
    per-core variance over angles for its 1/8 of bins -> [16, 512].
    Squares on the Act engine overlap the sum chain on DVE.
  - The 5.9MB image-plane copy (DRAM->DRAM) starts right after the halo
    read and fully overlaps the GLCM loop, finishing before the
    collective needs the DMA rings.
"""
import sys

for _p in ("/opt/trn_rl_repo",):
    if _p not in sys.path:
        sys.path.insert(0, _p)

import numpy as np

import concourse.bass as bass
import concourse.mybir as mybir
from concourse import bacc, tile
from concourse.bass_utils import run_bass_kernel_spmd
from concourse.tile_rust import add_dep_helper

F32 = mybir.dt.float32
F16 = mybir.dt.float16
FP8 = mybir.dt.float8e4
I16 = mybir.dt.int16

N_CORES = 8
NPLANES = 180
H = W = 256
ROWS_PER_CORE = NPLANES * H // N_CORES  # 5760

PW = 258                  # padded row width
TCOLS = 528               # pair columns: 128 * 528 = 67584 >= 258*258
TPC = TCOLS // N_CORES    # 66 pair columns per core
RD_BASE = 462             # fixed halo read base; write base = 721 - 66*m
STG = 69120               # staging elements (= RD_BASE + 128*TCOLS + pad)
SENT = 300.0

_CACHED = {}


def build_nc():
    nc = bacc.Bacc("TRN2", target_bir_lowering=False, debug=False,
                   enable_asserts=False, num_devices=N_CORES)

    img = nc.declare_dram_parameter("img", [ROWS_PER_CORE, 256], F32,
                                    isOutput=False)
    band = nc.declare_dram_parameter("band", [128, 512], F32, isOutput=False)
    img_out = nc.declare_dram_parameter("img_out", [ROWS_PER_CORE, 256], F32,
                                        isOutput=True)
    var_out = nc.declare_dram_parameter("var_out", [16, 512], F32,
                                        isOutput=True)

    staging = nc.dram_tensor("staging", [STG], F32)
    cc_in = nc.dram_tensor("cc_in", [128 * 2048], F16)
    cc_out = nc.dram_tensor("cc_out", [16 * 2048], F16)


    with tile.TileContext(nc) as tc:
        with (
            tc.tile_pool(name="const", bufs=1) as cpool,
            tc.tile_pool(name="prep", bufs=1) as prep,
            tc.tile_pool(name="psum", bufs=1, space="PSUM") as psp,
            tc.tile_pool(name="post", bufs=1) as post,
        ):
            stg_flat = staging.ap()

            # ---- dependency-free constants / prefills ----
            band_t = prep.tile([128, 512], F32)
            nc.sync.dma_start(out=band_t[0:64, :], in_=band[0:64, :])
            nc.gpsimd.dma_start(out=band_t[64:128, :], in_=band[64:128, :])

            sent4 = prep.tile([4, 736], F32)
            nc.vector.memset(sent4[:], SENT)
            # head [0, 736) and tail [66176, 69120) sentinel prefill; the
            # pixel write (base = 721-66m, 66048 long) lands inside and
            # overwrites the middle.
            nc.scalar.dma_start(out=stg_flat[0:736].rearrange(
                "(p f) -> p f", p=1), in_=sent4[0:1, :])
            nc.scalar.dma_start(out=stg_flat[66176:69120].rearrange(
                "(p f) -> p f", p=4), in_=sent4[:, :])

            iota16 = cpool.tile([128, 256], I16)
            nc.gpsimd.iota(iota16[:], pattern=[[1, 256]], base=0,
                           channel_multiplier=0)
            pid16 = cpool.tile([128, 1], I16)
            nc.gpsimd.iota(pid16[:], pattern=[[1, 1]], base=0,
                           channel_multiplier=1)
            pidf = cpool.tile([128, 1], F32)
            nc.vector.tensor_copy(pidf[:], pid16[:])
            ident = cpool.tile([128, 128], F32)
            nc.vector.tensor_scalar(ident[:], iota16[:, 0:128], pidf[:],
                                    None, mybir.AluOpType.is_equal)
            ones1 = cpool.tile([1, 128], F32)
            nc.vector.memset(ones1[:], 1.0)

            # ---- quantize band to u8 (identical on every core) ----
            mn = prep.tile([128, 1], F32)
            mx2 = prep.tile([128, 2], F32)  # [:,0] = -lo_p, [:,1] = hi_p
            nc.vector.tensor_reduce(mn[:], band_t[:], mybir.AxisListType.X,
                                    mybir.AluOpType.min)
            nc.vector.tensor_reduce(mx2[:, 1:2], band_t[:],
                                    mybir.AxisListType.X, mybir.AluOpType.max)
            nc.vector.tensor_scalar(mx2[:, 0:1], mn[:], -1.0, None,
                                    mybir.AluOpType.mult)
            # cross-partition max via PE transpose -> reduce -> transpose
            # -> ones-matmul broadcast
            psQ = psp.tile([128, 512], F32, name="psQ", tag="psQ")
            nc.tensor.transpose(psQ[0:2, 0:128], mx2[:], ident[:])
            sT = prep.tile([2, 128], F32)
            nc.scalar.copy(sT[:], psQ[0:2, 0:128])
            t2 = prep.tile([2, 1], F32)
            nc.vector.tensor_reduce(t2[:], sT[:], mybir.AxisListType.X,
                                    mybir.AluOpType.max)
            nc.tensor.transpose(psQ[0:1, 128:130], t2[:], ident[0:2, 0:2])
            sT2 = prep.tile([1, 2], F32)
            nc.vector.tensor_copy(sT2[:], psQ[0:1, 128:130])
            nc.tensor.matmul(psQ[:, 130:132], ones1[:], sT2[:], start=True,
                             stop=True)
            pmax = prep.tile([128, 2], F32)  # [:,0] = -lo, [:,1] = hi
            nc.vector.tensor_copy(pmax[:], psQ[:, 130:132])

            den = prep.tile([128, 1], F32)
            nc.vector.tensor_tensor(den[:], pmax[:, 1:2], pmax[:, 0:1],
                                    mybir.AluOpType.add)  # hi - lo
            nc.vector.tensor_scalar(den[:], den[:], 1e-12, None,
                                    mybir.AluOpType.max)
            rcp = prep.tile([128, 1], F32)
            nc.vector.reciprocal(rcp[:], den[:])
            nc.vector.tensor_scalar(rcp[:], rcp[:], 255.0, None,
                                    mybir.AluOpType.mult)
            scaled = prep.tile([128, 512], F32)
            nc.vector.tensor_scalar(scaled[:], band_t[:], pmax[:, 0:1], None,
                                    mybir.AluOpType.add)      # band - lo
            nc.vector.tensor_scalar(scaled[:], scaled[:], rcp[:], None,
                                    mybir.AluOpType.mult)     # * 255/(hi-lo)

            # ---- u8 quantize straight into the SBUF-padded layout ----
            u8p = prep.tile([128, 516], F32)  # [128, 2, 258] padded rows
            u8v = u8p[:].rearrange("p (k c) -> p k c", c=PW)
            nc.vector.memset(u8v[:, :, 256:258], SENT)
            # round-to-nearest-even via the fp32 magic constant: for
            # 0 <= x < 2^22, (x + 1.5*2^23) - 1.5*2^23 == round(x)
            MAGIC = 12582912.0
            nc.vector.tensor_scalar(u8v[:, :, 0:256], scaled[:], MAGIC,
                                    -MAGIC, mybir.AluOpType.add,
                                    mybir.AluOpType.add)

            # ---- staging: static contiguous write, static overlapped-row
            # read, per-core column shift as a register-offset DVE copy ----
            # write pixels at stg[259 + 258r + c]; core m's halo column j of
            # partition p is stg[528p + 66m + j], so a static [128, 990]
            # read (partition stride 528, overlapping rows) covers all
            # cores and the shift is a 66m column offset in SBUF.
            wr2d = stg_flat[259:259 + 128 * 516].rearrange(
                "(p f) -> p f", f=516)
            for eng, lo, hi in ((nc.sync, 0, 48), (nc.scalar, 48, 88),
                                (nc.gpsimd, 88, 128)):
                eng.dma_start(out=wr2d[lo:hi, :], in_=u8p[lo:hi, :])
            # two 2D reads with width <= pitch (the 990-wide window's rows
            # overlap by 462, which would break 2D-descriptor mode), each
            # split across two queues
            wide = prep.tile([128, 990], F32)
            rd_dmas = [
                nc.sync.dma_start(
                    out=wide[0:64, 0:528],
                    in_=bass.AP(stg_flat.tensor, 0, [[528, 64], [1, 528]])),
                nc.scalar.dma_start(
                    out=wide[64:128, 0:528],
                    in_=bass.AP(stg_flat.tensor, 528 * 64,
                                [[528, 64], [1, 528]])),
                nc.gpsimd.dma_start(
                    out=wide[:, 528:990],
                    in_=bass.AP(stg_flat.tensor, 528,
                                [[528, 128], [1, 462]])),
            ]
            pidv = nc.vector.partition_id()
            halo = prep.tile([128, 528], F32)
            nc.vector.tensor_copy(halo[:], wide[:, bass.ds(TPC * pidv, 528)])

            # ---- big image copy (DRAM -> DRAM) ----
            # Held back until the halo read completes so the staging chain
            # never queues behind it; it then overlaps the GLCM loop and
            # drains before the collective needs the DMA rings.
            chunk = ROWS_PER_CORE // 4
            for c in range(4):
                eng = nc.scalar if c < 2 else nc.gpsimd
                cp = eng.dma_start(
                    out=img_out[c * chunk:(c + 1) * chunk, :],
                    in_=img[c * chunk:(c + 1) * chunk, :],
                )
                for rd in rd_dmas:
                    add_dep_helper(cp.ins, rd.ins, sync=True,
                                   reason="image copy after GLCM prep DMAs")

            # ---- GLCM one-hot matmuls (fp8 DoubleRow; fp16 build trick) ----
            # stream[:, s, :] (fp16) = (iota == halo col(s)): col(s) = s for
            # s <= 66 (A/B0 roles), col(s) = s+190 for s >= 67 (cols
            # 257..324, B1/B2/B3 roles).  fp16 1.0 = 0x3C00, so through the
            # fp8 bitcast view the ODD bytes are a one-hot with value 1.5
            # (e4m3 0x3C); products come out scaled by 2.25, undone at the
            # PSUM drain.  Keeping every DVE operand 2-byte and the op
            # single-ALU gives the builds a shot at 4x DVE perf mode.
            # Roles per column t: A=slot t, B0=t+1, B3=t+67, B2=t+68,
            # B1=t+69.
            DR = mybir.MatmulPerfMode.DoubleRow
            ps0 = psp.tile([128, 512], F32, name="ps0", tag="ps0")
            ps1a = psp.tile([128, 256], F32, name="ps1a", tag="ps1a")
            ps1b = psp.tile([128, 256], F32, name="ps1b", tag="ps1b")
            ps2 = psp.tile([128, 512], F32, name="ps2", tag="ps2")
            ps3a = psp.tile([128, 256], F32, name="ps3a", tag="ps3a")
            ps3b = psp.tile([128, 256], F32, name="ps3b", tag="ps3b")
            stream = cpool.tile([128, 135, 256], F16)

            def build(slot, col):
                nc.vector.tensor_scalar(
                    stream[:, slot, :], iota16[:], halo[:, col:col + 1],
                    None, mybir.AluOpType.is_equal)

            build(0, 0)
            build(67, 257)
            build(68, 258)
            st8 = stream[:].bitcast(FP8)  # [128, 135, 512] fp8 view
            pd8 = list(st8.ap[0])
            t8 = st8.tensor

            def mk(slot0, off, n):
                # [K = 128p x 2 ktiles, N = n] stride-2 odd-byte view;
                # ktile k is slot slot0+k.
                return bass.AP(t8, st8.offset + slot0 * 512 + off + 1,
                               [pd8, [512, 2], [2, n]])

            for tt in range(0, TPC, 2):
                build(tt + 1, tt + 1)
                build(tt + 2, tt + 2)
                build(tt + 69, tt + 259)
                build(tt + 70, tt + 260)
                st, sp = (tt == 0), (tt == TPC - 2)
                a_lo = mk(tt, 0, 128)
                a_hi = mk(tt, 256, 128)
                r32 = mk(tt + 67, 0, 512)   # [B3 | B2] for pairs (t, t+1)
                rb1 = mk(tt + 69, 0, 256)
                rb0 = mk(tt + 1, 0, 256)
                nc.tensor.matmul(ps0[:], a_lo, r32, start=st, stop=sp,
                                 perf_mode=DR)
                nc.tensor.matmul(ps1a[:], a_lo, rb1, start=st, stop=sp,
                                 perf_mode=DR)
                nc.tensor.matmul(ps1b[:], a_lo, rb0, start=st, stop=sp,
                                 perf_mode=DR)
                nc.tensor.matmul(ps2[:], a_hi, r32, start=st, stop=sp,
                                 perf_mode=DR)
                nc.tensor.matmul(ps3a[:], a_hi, rb1, start=st, stop=sp,
                                 perf_mode=DR)
                nc.tensor.matmul(ps3b[:], a_hi, rb0, start=st, stop=sp,
                                 perf_mode=DR)

            # ---- counts -> DRAM -> ReduceScatter ----
            # counts_sb[l, 1024*h + 256*k + j] = counts[level 128*h+l, j, k];
            # the 1/2.25 undoes the 1.5*1.5 one-hot product scale.
            DS = 1.0 / 2.25
            counts_sb = post.tile([128, 2048], F16)
            nc.scalar.mul(counts_sb[:, 0:512], ps0[:], DS)
            nc.vector.tensor_scalar(counts_sb[:, 512:768], ps1a[:], DS, None,
                                    mybir.AluOpType.mult)
            nc.vector.tensor_scalar(counts_sb[:, 768:1024], ps1b[:], DS, None,
                                    mybir.AluOpType.mult)
            nc.scalar.mul(counts_sb[:, 1024:1536], ps2[:], DS)
            nc.vector.tensor_scalar(counts_sb[:, 1536:1792], ps3a[:], DS,
                                    None, mybir.AluOpType.mult)
            nc.vector.tensor_scalar(counts_sb[:, 1792:2048], ps3b[:], DS,
                                    None, mybir.AluOpType.mult)
            cc2d = cc_in.ap().rearrange("(p f) -> p f", p=128)
            nc.sync.dma_start(out=cc2d[0:64, :], in_=counts_sb[0:64, :])
            nc.scalar.dma_start(out=cc2d[64:128, :], in_=counts_sb[64:128, :])
            nc.gpsimd.collective_compute(
                "ReduceScatter",
                mybir.AluOpType.add,
                replica_groups=[list(range(N_CORES))],
                ins=[cc_in.ap().opt()],
                outs=[cc_out.ap().opt()],
            )
            c16 = post.tile([16, 2048], F16)
            nc.sync.dma_start(out=c16[:],
                              in_=cc_out.ap().rearrange("(p f) -> p f", p=16))

            # ---- variance over the 4 angles ----
            # squares on the Act engine overlap the sum chain on DVE
            c3 = c16[:].rearrange("p (h k j) -> p h k j", h=2, k=4)
            csq = post.tile([16, 2048], F32)
            q3 = csq[:].rearrange("p (h k j) -> p h k j", h=2, k=4)
            nc.scalar.square(csq[:], c16[:])
            s = post.tile([16, 512], F32)
            q = post.tile([16, 512], F32)
            s2 = s[:].rearrange("p (h j) -> p h j", h=2)
            q2 = q[:].rearrange("p (h j) -> p h j", h=2)
            nc.vector.tensor_tensor(s2[:, :, :], c3[:, :, 0, :],
                                    c3[:, :, 1, :], mybir.AluOpType.add)
            nc.vector.tensor_tensor(s2[:, :, :], s2[:, :, :], c3[:, :, 2, :],
                                    mybir.AluOpType.add)
            nc.vector.tensor_tensor(s2[:, :, :], s2[:, :, :], c3[:, :, 3, :],
                                    mybir.AluOpType.add)
            nc.vector.tensor_tensor(q2[:, :, :], q3[:, :, 0, :],
                                    q3[:, :, 1, :], mybir.AluOpType.add)
            nc.vector.tensor_tensor(q2[:, :, :], q2[:, :, :], q3[:, :, 2, :],
                                    mybir.AluOpType.add)
            nc.vector.tensor_tensor(q2[:, :, :], q2[:, :, :], q3[:, :, 3, :],
                                    mybir.AluOpType.add)
            # var = q/4 - (s/16)*s
            tmp = post.tile([16, 512], F32)
            nc.vector.scalar_tensor_tensor(tmp[:], s[:], 0.0625, s[:],
                                           mybir.AluOpType.mult,
                                           mybir.AluOpType.mult)
            var_t = post.tile([16, 512], F32)
            nc.vector.scalar_tensor_tensor(var_t[:], q[:], 0.25, tmp[:],
                                           mybir.AluOpType.mult,
                                           mybir.AluOpType.subtract)
            nc.sync.dma_start(out=var_out[:], in_=var_t[:])

    nc.compile()
    return nc


def get_nc():
    if "nc" not in _CACHED:
        _CACHED["nc"] = build_nc()
    return _CACHED["nc"]


def make_in_maps(image, band):
    flat = image.reshape(NPLANES * H, W)
    band2 = np.ascontiguousarray(band.reshape(128, 512))
    return [
        {
            "img": np.ascontiguousarray(
                flat[m * ROWS_PER_CORE:(m + 1) * ROWS_PER_CORE]),
            "band": band2,
        }
        for m in range(N_CORES)
    ]


def assemble(image_shards, var_shards):
    """image_shards: 8 x [5760,256]; var_shards: 8 x [16,512] -> [181,256,256]."""
    out = np.empty((NPLANES + 1, H, W), dtype=np.float32)
    out[:NPLANES] = np.concatenate(image_shards, axis=0).reshape(NPLANES, H, W)
    var = out[NPLANES]
    for m in range(N_CORES):
        v = var_shards[m]
        var[16 * m:16 * m + 16, :] = v[:, 0:256]
        var[128 + 16 * m:128 + 16 * m + 16, :] = v[:, 256:512]
    return out


def kernel(image, index):
    image = np.ascontiguousarray(np.asarray(image, dtype=np.float32))
    idx = int(np.asarray(index))
    band = image[idx]

    nc = get_nc()
    in_maps = make_in_maps(image, band)
    last_err = None
    for attempt in range(3):
        try:
            res = run_bass_kernel_spmd(nc, in_maps,
                                       core_ids=list(range(N_CORES)))
            break
        except Exception as e:  # transient NRT device errors
            last_err = e
            import time
            time.sleep(15)
    else:
        raise last_err
    return assemble(
        [res.results[m]["img_out"] for m in range(N_CORES)],
        [res.results[m]["var_out"] for m in range(N_CORES)],
    )
